# revision 48
# baseline (speedup 1.0000x reference)
"""MiniTransformerBlock on 8 TRN2 NeuronCores (Bass/Tile), sequence-parallel.

Reference computation (S=4096, D=1024, V=32000):
    h = emb[x]                                  # [S, D]
    h = h * rsqrt(mean(h^2, -1) + eps) * norm_w # RMSNorm
    q, k, v = h @ Wq.T, h @ Wk.T, h @ Wv.T
    out = silu(softmax(q @ k.T) @ v)            # [S, D]  (no scale, no mask)

v4 ("hT-gather", software-pipelined). Key ideas vs the v2 baseline
(258 us):
  - softmax(q k^T) v = softmax((h Wq^T Wk) h^T) h Wv^T: gather ONLY the
    normalized h^T (8 MB fp16) instead of k^T + v (16 MB) -- HW A/B
    showed the two serialized v2 AllGathers exposed ~134 us.
  - flash softmax: per-512-chunk max + exp straight from PSUM into fp16,
    global fixup exp(m_c - m_g)/rowsum applied as one per-partition
    scalar per chunk.  No 8 MB f32 score buffer, no serial exp phase.
  - row-major h (for attn@h) is produced by PE-transposing the kc score
    tiles while they are in SBUF -- no extra HBM traffic.  (The DMA
    XBAR transpose is bit-correct in isolation but races with the
    collectives' SDMA fabric when reps pipeline, so it is not used.)
  - software-pipelined emission: rep r+1's head (embedding gather,
    RMSNorm on ACT/Pool, h^T transposes, AllGather issue) is emitted in
    the middle of rep r's body so the gather wire time hides under
    rep r's attn@h and rep r+1's projections.
  - weights live in SBUF (loaded once per call); all PSUM->SBUF copies
    on DVE/Pool so ACT only runs exp/silu; score chunks are 512 wide to
    halve per-instruction overheads.
"""

import os

import numpy as np

import concourse.bacc as bacc
import concourse.bass as bass
import concourse.tile as tile
from concourse import mybir
from concourse.bass_utils import run_bass_kernel_spmd
from concourse.masks import make_identity

P = 128
S = 4096
D = 1024
V = 32000
NCORES = 8
SL = S // NCORES          # 512 local rows
TLOC = SL // P            # 4 local row tiles
DC = D // P               # 8 feature chunks
JC = NCORES               # 8 key chunks of 512 (one per source core)
JB = S // P               # 32 key row blocks
HS = SL // 2              # 256 seq half (gather split granularity)
F32 = mybir.dt.float32
F16 = mybir.dt.float16
EPS = float(np.finfo(np.float32).eps)

_cache = {}

MODE = os.environ.get("BASS_MODE", "full")   # full | noag
REPS = int(os.environ.get("BASS_REPS", "1"))
NOSILU = os.environ.get("BASS_NOSILU", "0") == "1"  # CoreSim lacks Silu


def build(reps=None):
    if reps is None:
        reps = REPS
    nc = bacc.Bacc("TRN2", target_bir_lowering=False, debug=False,
                   num_devices=NCORES)

    x_loc = nc.dram_tensor("x_loc", [SL, 1], mybir.dt.int32, kind="ExternalInput")
    emb = nc.dram_tensor("emb", [V, D], F32, kind="ExternalInput")
    norm_w = nc.dram_tensor("norm_w", [D], F32, kind="ExternalInput")
    # host preps: wm[d,o] = (Wq.T @ Wk)[d,o];  wvt[d,o] = Wv[o,d]
    wm = nc.dram_tensor("wm", [D, D], F16, kind="ExternalInput")
    wvt = nc.dram_tensor("wvt", [D, D], F16, kind="ExternalInput")
    out_loc = nc.dram_tensor("out_loc", [SL, D], F32, kind="ExternalOutput")

    with tile.TileContext(nc) as tc:
        Pipe(nc, tc, x_loc, emb, norm_w, wm, wvt, out_loc).emit(reps)
    nc.compile()
    return nc


class Pipe:
    def __init__(self, nc, tc, x_loc, emb, norm_w, wm, wvt, out_loc):
        self.nc = nc
        self.tc = tc
        self.x_loc = x_loc
        self.emb = emb
        self.norm_w = norm_w
        self.wm = wm
        self.wvt = wvt
        self.out_loc = out_loc
        self.state = {}

    def emit(self, reps):
        nc, tc = self.nc, self.tc
        with (
            tc.tile_pool(name="const", bufs=1) as const,
            tc.tile_pool(name="dram", bufs=1, space="DRAM") as dram,
            tc.tile_pool(name="hp", bufs=1) as hp,
            tc.tile_pool(name="scratch", bufs=1) as scratch,
            tc.tile_pool(name="stats", bufs=1) as stats,
            tc.tile_pool(name="htp", bufs=1) as htp,
            tc.tile_pool(name="pst", bufs=2, space="PSUM") as pst,
            tc.tile_pool(name="q2p", bufs=1) as q2p,
            tc.tile_pool(name="ostats", bufs=1) as ostats,
            tc.tile_pool(name="atp", bufs=1) as atp,
            tc.tile_pool(name="hrmp", bufs=1) as hrmp,
            tc.tile_pool(name="ohp", bufs=1) as ohp,
        ):
            self.dram = dram
            self.hp = hp
            self.scratch = scratch
            self.stats = stats
            self.htp = htp
            self.pst = pst
            self.q2p = q2p
            self.ostats = ostats
            self.atp = atp
            self.hrmp = hrmp
            self.ohp = ohp

            ident_f = const.tile([P, P], F32)
            make_identity(nc, ident_f[:])
            self.ident_h = const.tile([P, P], F16)
            nc.vector.tensor_copy(self.ident_h[:], ident_f[:])
            self.eps_t = const.tile([P, 1], F32)
            nc.vector.memset(self.eps_t[:], EPS)
            self.w_cols = const.tile([P, DC], F32)
            nc.sync.dma_start(
                out=self.w_cols[:],
                in_=self.norm_w.ap().rearrange("(a b) -> b a", b=P))
            self.x_sb = const.tile([P, TLOC], mybir.dt.int32)
            nc.sync.dma_start(
                out=self.x_sb[:],
                in_=self.x_loc.ap().rearrange("(a b) c -> b (a c)", b=P))
            # q-path weight (Wq.T @ Wk, host-premultiplied) resident in SBUF
            self.wm_sb = const.tile([P, DC, D], F16)
            nc.sync.dma_start(
                out=self.wm_sb[:],
                in_=self.wm.ap().rearrange("(a b) c -> b a c", b=P))

            self.head(0)
            self.head_gather(0)
            for rep in range(reps):
                self.body(rep, reps)

    # ---- head part A: emb gather + RMSNorm (ACT/DVE/Pool only) ----
    def head(self, rep):
        nc = self.nc
        ht_in = [self.dram.tile([D, HS], F16, tag=f"ht_in{rep}h{h}",
                                name=f"ht_in{rep}h{h}") for h in range(2)]
        ht_out = [self.dram.tile([NCORES * D, HS], F16,
                                 tag=f"ht_out{rep}h{h}",
                                 name=f"ht_out{rep}h{h}",
                                 addr_space="Shared") for h in range(2)]
        hn = []
        for t in range(TLOC):
            ht = self.hp.tile([P, D], F32, tag=f"h{t % 2}",
                              name=f"h{t}_{rep}")
            nc.gpsimd.indirect_dma_start(
                out=ht[:], out_offset=None, in_=self.emb[:, :],
                in_offset=bass.IndirectOffsetOnAxis(
                    ap=self.x_sb[:, t:t + 1], axis=0),
            )
            sq = self.scratch.tile([P, D], F32, tag="sq",
                                   name=f"sq{t}_{rep}")
            ss = self.stats.tile([P, 1], F32, tag=f"ss{t}",
                                 name=f"ss{t}_{rep}")
            nc.scalar.activation(
                out=sq[:], in_=ht[:],
                func=mybir.ActivationFunctionType.Square, accum_out=ss[:])
            sd = self.stats.tile([P, 1], F32, tag=f"sd{t}",
                                 name=f"sd{t}_{rep}")
            nc.scalar.activation(
                out=sd[:], in_=ss[:],
                func=mybir.ActivationFunctionType.Sqrt,
                bias=self.eps_t[:], scale=1.0 / D)
            rv = self.stats.tile([P, 1], F32, tag=f"rv{t}",
                                 name=f"rv{t}_{rep}")
            nc.vector.reciprocal(rv[:], sd[:])
            hh = self.hp.tile([P, D], F16, tag=f"hn{t}", name=f"hn{t}_{rep}")
            # Pool engine: keeps DVE free for the surrounding body's copies
            nc.gpsimd.tensor_scalar_mul(out=hh[:], in0=ht[:], scalar1=rv[:])
            hn.append(hh)
        self.state[rep] = dict(ht_in=ht_in, ht_out=ht_out, hn=hn)

    # ---- head part B: h^T transposes + gather issue (PE/Pool/DVE/SP) ----
    def head_gather(self, rep):
        nc = self.nc
        st = self.state[rep]
        hn = st.pop("hn")
        ht_in = st["ht_in"]
        hTr = []
        for dc in range(DC):
            pt = self.pst.tile([P, SL], F16, tag="pt", name=f"pt{dc}_{rep}")
            for t in range(TLOC):
                nc.tensor.transpose(
                    pt[:, t * P:(t + 1) * P],
                    in_=hn[t][:, dc * P:(dc + 1) * P],
                    identity=self.ident_h[:])
            htr = self.htp.tile([P, SL], F16, tag=f"htr{dc}",
                                name=f"htr{dc}_{rep}")
            # GPSIMD cannot read PSUM -- DVE only here
            nc.vector.tensor_scalar_mul(
                out=htr[:], in0=pt[:], scalar1=self.w_cols[:, dc:dc + 1])
            hTr.append(htr)
            nc.sync.dma_start(
                out=ht_in[0][dc * P:(dc + 1) * P, :], in_=htr[:, 0:HS])
            nc.sync.dma_start(
                out=ht_in[1][dc * P:(dc + 1) * P, :], in_=htr[:, HS:SL])
        if MODE == "full":
            for h in range(2):
                nc.gpsimd.collective_compute(
                    "AllGather", mybir.AluOpType.bypass,
                    replica_groups=[list(range(NCORES))],
                    ins=[ht_in[h][:].opt()],
                    outs=[st["ht_out"][h][:].opt()])
        st["hTr"] = hTr

    def body(self, rep, reps):
        nc, tc = self.nc, self.tc
        st = self.state[rep]
        ht_in, ht_out = st["ht_in"], st["ht_out"]
        hTr = st["hTr"]

        # ---- q'' = h @ (Wq^T Wk) ----
        q2t = []
        with tc.tile_pool(name="psq", bufs=2, space="PSUM") as psq:
            for mo in range(DC):
                pp = psq.tile([P, SL], F32, tag="pp", name=f"pp{mo}_{rep}")
                for dc in range(DC):
                    nc.tensor.matmul(
                        pp[:], self.wm_sb[:, dc, mo * P:(mo + 1) * P],
                        hTr[dc][:], start=(dc == 0), stop=(dc == DC - 1))
                xt = self.q2p.tile([P, SL], F16, tag=f"q2t{mo}",
                                   name=f"q2t{mo}_{rep}")
                nc.vector.tensor_copy(xt[:], pp[:])
                q2t.append(xt)

        # ---- scores + flash softmax + row-major h production ----
        nm = [self.ostats.tile([P, JC], F32, tag=f"nm{t}", name=f"nm{t}_{rep}")
              for t in range(TLOC)]
        rs = [self.ostats.tile([P, JC], F32, tag=f"rs{t}", name=f"rs{t}_{rep}")
              for t in range(TLOC)]
        aT = [self.atp.tile([P, TLOC, P], F16, tag=f"aT{jb}",
                            name=f"aT{jb}_{rep}") for jb in range(JB)]
        hrm = [None] * JB
        with tc.tile_pool(name="ep", bufs=1) as ep:
            e = [ep.tile([P, S], F16, tag=f"e{t}", name=f"e{t}_{rep}")
                 for t in range(TLOC)]
            with (
                tc.tile_pool(name="kcp", bufs=2) as kcp,
                tc.tile_pool(name="pss", bufs=4, space="PSUM") as pss,
                tc.tile_pool(name="pth", bufs=2, space="PSUM") as pth,
            ):
                for jc in range(JC):
                    kc = kcp.tile([P, DC, SL], F16, tag="kc",
                                  name=f"kc{jc}_{rep}")
                    for h in range(2):
                        src = (ht_in[h][:, :] if MODE == "noag"
                               else ht_out[h][jc * D:(jc + 1) * D, :])
                        nc.sync.dma_start(
                            out=kc[:, :, h * HS:(h + 1) * HS],
                            in_=src.rearrange("(a b) c -> b a c", b=P))
                    for t in range(TLOC):
                        ps = pss.tile([P, SL], F32, tag="ps",
                                      name=f"ps{jc}_{t}_{rep}")
                        for dc in range(DC):
                            nc.tensor.matmul(
                                ps[:], q2t[dc][:, t * P:(t + 1) * P],
                                kc[:, dc, :],
                                start=(dc == 0), stop=(dc == DC - 1))
                        nc.vector.reduce_max(
                            out=nm[t][:, jc:jc + 1], in_=ps[:],
                            axis=mybir.AxisListType.X, negate=True)
                        nc.scalar.activation(
                            out=e[t][:, jc * SL:(jc + 1) * SL], in_=ps[:],
                            func=mybir.ActivationFunctionType.Exp,
                            bias=nm[t][:, jc:jc + 1], scale=1.0,
                            accum_out=rs[t][:, jc:jc + 1])
                    # transpose this chunk of h^T into row-major h while
                    # it's in SBUF (value-side operand of attn@h)
                    for c2 in range(TLOC):
                        jb = jc * TLOC + c2
                        ph = pth.tile([P, D], F16, tag="ph",
                                      name=f"ph{jb}_{rep}")
                        for dc in range(DC):
                            nc.tensor.transpose(
                                ph[:, dc * P:(dc + 1) * P],
                                in_=kc[:, dc, c2 * P:(c2 + 1) * P],
                                identity=self.ident_h[:])
                        hm = self.hrmp.tile([P, D], F16, tag=f"hrm{jb}",
                                            name=f"hrm{jb}_{rep}")
                        # GPSIMD cannot read PSUM: 3 copies DVE, 1 ACT
                        if c2 == 3:
                            nc.scalar.activation(
                                out=hm[:], in_=ph[:],
                                func=mybir.ActivationFunctionType.Copy)
                        else:
                            nc.vector.tensor_copy(hm[:], ph[:])
                        hrm[jb] = hm

            # ---- fixup factors: exp(m_c - m_g)/rowsum, then e *= g ----
            with tc.tile_pool(name="fxp", bufs=1) as fxp:
                for t in range(TLOC):
                    gneg = fxp.tile([P, 1], F32, tag=f"gneg{t}",
                                    name=f"gneg{t}_{rep}")
                    nc.vector.tensor_reduce(
                        out=gneg[:], in_=nm[t][:],
                        axis=mybir.AxisListType.X, op=mybir.AluOpType.min)
                    f = fxp.tile([P, JC], F32, tag=f"f{t}",
                                 name=f"f{t}_{rep}")
                    nc.scalar.activation(
                        out=f[:], in_=nm[t][:],
                        func=mybir.ActivationFunctionType.Exp,
                        bias=gneg[:], scale=-1.0)
                    wr = fxp.tile([P, JC], F32, tag=f"wr{t}",
                                  name=f"wr{t}_{rep}")
                    nc.vector.tensor_tensor(
                        out=wr[:], in0=rs[t][:], in1=f[:],
                        op=mybir.AluOpType.mult)
                    rowsum = fxp.tile([P, 1], F32, tag=f"rsum{t}",
                                      name=f"rsum{t}_{rep}")
                    nc.vector.reduce_sum(
                        out=rowsum[:], in_=wr[:], axis=mybir.AxisListType.X)
                    rinv = fxp.tile([P, 1], F32, tag=f"rinv{t}",
                                    name=f"rinv{t}_{rep}")
                    nc.vector.reciprocal(rinv[:], rowsum[:])
                    g = fxp.tile([P, JC], F32, tag=f"g{t}",
                                 name=f"g{t}_{rep}")
                    nc.vector.tensor_scalar_mul(
                        out=g[:], in0=f[:], scalar1=rinv[:])
                    for jc in range(JC):
                        # e is SBUF, so GPSIMD may help drain the fixup
                        eng = nc.vector if jc % 2 == 0 else nc.gpsimd
                        eng.tensor_scalar_mul(
                            out=e[t][:, jc * SL:(jc + 1) * SL],
                            in0=e[t][:, jc * SL:(jc + 1) * SL],
                            scalar1=g[:, jc:jc + 1])

            # next rep's head part A -- runs on ACT/DVE/Pool while this
            # rep's transposes and attn@h occupy the PE
            if rep + 1 < reps:
                self.head(rep + 1)

            # ---- aT transposes ----
            with tc.tile_pool(name="ptp", bufs=2, space="PSUM") as ptp:
                for jb in range(JB):
                    pt2 = ptp.tile([P, TLOC, P], F16, tag="pt2",
                                   name=f"pt2{jb}_{rep}")
                    for t in range(TLOC):
                        nc.tensor.transpose(
                            pt2[:, t, :],
                            in_=e[t][:, jb * P:(jb + 1) * P],
                            identity=self.ident_h[:])
                    if jb % 2 == 0:
                        nc.vector.tensor_copy(aT[jb][:], pt2[:])
                    else:
                        nc.scalar.activation(
                            out=aT[jb][:], in_=pt2[:],
                            func=mybir.ActivationFunctionType.Copy)

        # next rep's head part B: h^T transposes on PE + AllGather issue --
        # the gather's wire time hides under this rep's attn@h + final
        if rep + 1 < reps:
            self.head_gather(rep + 1)

        # ---- attn @ h (output d-major), two passes of 4 PSUM banks ----
        ohT = [None] * DC
        with (
            tc.tile_pool(name="wvp", bufs=1) as wvp,
            tc.tile_pool(name="pvp", bufs=1, space="PSUM") as pvp,
        ):
            wv_sb = wvp.tile([P, DC, D], F16, name=f"wv_sb_{rep}")
            nc.scalar.dma_start(
                out=wv_sb[:],
                in_=self.wvt.ap().rearrange("(a b) c -> b a c", b=P))
            for dcg in range(2):
                po = [pvp.tile([P, TLOC, P], F32, tag=f"po{i}",
                               name=f"po{i}_{dcg}_{rep}") for i in range(4)]
                for jb in range(JB):
                    for i in range(4):
                        dc = dcg * 4 + i
                        nc.tensor.matmul(
                            po[i][:], hrm[jb][:, dc * P:(dc + 1) * P],
                            aT[jb][:],
                            start=(jb == 0), stop=(jb == JB - 1))
                for i in range(4):
                    dc = dcg * 4 + i
                    oh = self.ohp.tile([P, TLOC, P], F16, tag=f"ohT{dc}",
                                       name=f"ohT{dc}_{rep}")
                    if i % 2 == 0:
                        nc.vector.tensor_copy(oh[:], po[i][:])
                    else:
                        nc.scalar.activation(
                            out=oh[:], in_=po[i][:],
                            func=mybir.ActivationFunctionType.Copy)
                    ohT[dc] = oh

            # ---- out = silu(out_h @ Wv^T) ----
            with (
                tc.tile_pool(name="outp", bufs=2) as outp,
                tc.tile_pool(name="pso", bufs=1, space="PSUM") as pso,
            ):
                for t in range(TLOC):
                    op = pso.tile([P, 2, 512], F32, tag="op",
                                  name=f"op{t}_{rep}")
                    for oh2 in range(2):
                        for dc in range(DC):
                            nc.tensor.matmul(
                                op[:, oh2, :], ohT[dc][:, t, :],
                                wv_sb[:, dc, oh2 * 512:(oh2 + 1) * 512],
                                start=(dc == 0), stop=(dc == DC - 1))
                    ot = outp.tile([P, 2, 512], F32, tag="ot",
                                   name=f"ot{t}_{rep}")
                    nc.scalar.activation(
                        out=ot[:], in_=op[:],
                        func=(mybir.ActivationFunctionType.Copy if NOSILU
                              else mybir.ActivationFunctionType.Silu))
                    nc.sync.dma_start(
                        out=self.out_loc[t * P:(t + 1) * P, :], in_=ot[:])


def kernel(x, emb, norm_w, Wq, Wk, Wv):
    if "nc" not in _cache:
        _cache["nc"] = build()
    nc = _cache["nc"]

    x = np.asarray(x).reshape(S).astype(np.int32)
    emb = np.ascontiguousarray(np.asarray(emb, dtype=np.float32))
    norm_w = np.ascontiguousarray(np.asarray(norm_w, dtype=np.float32))
    wm = np.ascontiguousarray(
        (np.asarray(Wq, np.float32).T @ np.asarray(Wk, np.float32))
        .astype(np.float16))
    wvt = np.ascontiguousarray(np.asarray(Wv, dtype=np.float16).T)

    in_maps = []
    for c in range(NCORES):
        in_maps.append({
            "x_loc": x[c * SL:(c + 1) * SL].reshape(SL, 1).copy(),
            "emb": emb, "norm_w": norm_w,
            "wm": wm, "wvt": wvt,
        })
    res = run_bass_kernel_spmd(nc, in_maps, core_ids=list(range(NCORES)),
                               **_cache.get("run_kwargs", {}))
    _cache["last_result"] = res
    out = np.concatenate([res.results[c]["out_loc"] for c in range(NCORES)],
                         axis=0)
    return out


# revision 49
# speedup vs baseline: 1.6670x; 1.6670x over previous
"""MiniTransformerBlock on 8 TRN2 NeuronCores (Bass/Tile), sequence-parallel.

Reference computation (S=4096, D=1024, V=32000):
    h = emb[x]                                  # [S, D]
    h = h * rsqrt(mean(h^2, -1) + eps) * norm_w # RMSNorm
    q, k, v = h @ Wq.T, h @ Wk.T, h @ Wv.T
    out = silu(softmax(q @ k.T) @ v)            # [S, D]  (no scale, no mask)

v4 ("hT-gather", software-pipelined). Key ideas vs the v2 baseline
(258 us):
  - softmax(q k^T) v = softmax((h Wq^T Wk) h^T) h Wv^T: gather ONLY the
    normalized h^T (8 MB fp16) instead of k^T + v (16 MB) -- HW A/B
    showed the two serialized v2 AllGathers exposed ~134 us.
  - flash softmax: per-512-chunk max + exp straight from PSUM into fp16,
    global fixup exp(m_c - m_g)/rowsum applied as one per-partition
    scalar per chunk.  No 8 MB f32 score buffer, no serial exp phase.
  - row-major h (for attn@h) is produced by PE-transposing the kc score
    tiles while they are in SBUF -- no extra HBM traffic.  (The DMA
    XBAR transpose is bit-correct in isolation but races with the
    collectives' SDMA fabric when reps pipeline, so it is not used.)
  - software-pipelined emission: rep r+1's head (embedding gather,
    RMSNorm on ACT/Pool, h^T transposes, AllGather issue) is emitted in
    the middle of rep r's body so the gather wire time hides under
    rep r's attn@h and rep r+1's projections.
  - weights live in SBUF (loaded once per call); all PSUM->SBUF copies
    on DVE/Pool so ACT only runs exp/silu; score chunks are 512 wide to
    halve per-instruction overheads.
"""

import os

import numpy as np

import concourse.bacc as bacc
import concourse.bass as bass
import concourse.tile as tile
from concourse import mybir
from concourse.bass_utils import run_bass_kernel_spmd
from concourse.masks import make_identity

P = 128
S = 4096
D = 1024
V = 32000
NCORES = 8
SL = S // NCORES          # 512 local rows
TLOC = SL // P            # 4 local row tiles
DC = D // P               # 8 feature chunks
JC = NCORES               # 8 key chunks of 512 (one per source core)
JB = S // P               # 32 key row blocks
HS = SL // 2              # 256 seq half (gather split granularity)
F32 = mybir.dt.float32
F16 = mybir.dt.float16
EPS = float(np.finfo(np.float32).eps)

_cache = {}

MODE = os.environ.get("BASS_MODE", "full")   # full | noag
REPS = int(os.environ.get("BASS_REPS", "1"))
NOSILU = os.environ.get("BASS_NOSILU", "0") == "1"  # CoreSim lacks Silu


def build(reps=None):
    if reps is None:
        reps = REPS
    nc = bacc.Bacc("TRN2", target_bir_lowering=False, debug=False,
                   num_devices=NCORES)

    x_loc = nc.dram_tensor("x_loc", [SL, 1], mybir.dt.int32, kind="ExternalInput")
    emb = nc.dram_tensor("emb", [V, D], F32, kind="ExternalInput")
    norm_w = nc.dram_tensor("norm_w", [D], F32, kind="ExternalInput")
    # host preps: wm[d,o] = (Wq.T @ Wk)[d,o];  wvt[d,o] = Wv[o,d]
    wm = nc.dram_tensor("wm", [D, D], F16, kind="ExternalInput")
    wvt = nc.dram_tensor("wvt", [D, D], F16, kind="ExternalInput")
    out_loc = nc.dram_tensor("out_loc", [SL, D], F32, kind="ExternalOutput")

    with tile.TileContext(nc) as tc:
        Pipe(nc, tc, x_loc, emb, norm_w, wm, wvt, out_loc).emit(reps)
    nc.compile()
    return nc


class Pipe:
    def __init__(self, nc, tc, x_loc, emb, norm_w, wm, wvt, out_loc):
        self.nc = nc
        self.tc = tc
        self.x_loc = x_loc
        self.emb = emb
        self.norm_w = norm_w
        self.wm = wm
        self.wvt = wvt
        self.out_loc = out_loc
        self.state = {}

    def emit(self, reps):
        nc, tc = self.nc, self.tc
        with (
            tc.tile_pool(name="const", bufs=1) as const,
            tc.tile_pool(name="dram", bufs=1, space="DRAM") as dram,
            tc.tile_pool(name="hp", bufs=1) as hp,
            tc.tile_pool(name="scratch", bufs=1) as scratch,
            tc.tile_pool(name="stats", bufs=1) as stats,
            tc.tile_pool(name="htp", bufs=1) as htp,
            tc.tile_pool(name="pst", bufs=2, space="PSUM") as pst,
            tc.tile_pool(name="q2p", bufs=1) as q2p,
            tc.tile_pool(name="ostats", bufs=1) as ostats,
            tc.tile_pool(name="atp", bufs=1) as atp,
            tc.tile_pool(name="hrmp", bufs=1) as hrmp,
            tc.tile_pool(name="ohp", bufs=1) as ohp,
        ):
            self.dram = dram
            self.hp = hp
            self.scratch = scratch
            self.stats = stats
            self.htp = htp
            self.pst = pst
            self.q2p = q2p
            self.ostats = ostats
            self.atp = atp
            self.hrmp = hrmp
            self.ohp = ohp

            ident_f = const.tile([P, P], F32)
            make_identity(nc, ident_f[:])
            self.ident_h = const.tile([P, P], F16)
            nc.vector.tensor_copy(self.ident_h[:], ident_f[:])
            self.eps_t = const.tile([P, 1], F32)
            nc.vector.memset(self.eps_t[:], EPS)
            self.w_cols = const.tile([P, DC], F32)
            nc.sync.dma_start(
                out=self.w_cols[:],
                in_=self.norm_w.ap().rearrange("(a b) -> b a", b=P))
            self.x_sb = const.tile([P, TLOC], mybir.dt.int32)
            nc.sync.dma_start(
                out=self.x_sb[:],
                in_=self.x_loc.ap().rearrange("(a b) c -> b (a c)", b=P))
            # q-path weight (Wq.T @ Wk, host-premultiplied) resident in SBUF
            self.wm_sb = const.tile([P, DC, D], F16)
            nc.sync.dma_start(
                out=self.wm_sb[:],
                in_=self.wm.ap().rearrange("(a b) c -> b a c", b=P))

            self.head(0)
            self.head_gather(0)
            for rep in range(reps):
                self.body(rep, reps)

    # ---- head part A: emb gather + RMSNorm (ACT/DVE/Pool only) ----
    def head(self, rep):
        nc = self.nc
        ht_in = [self.dram.tile([D, HS], F16, tag=f"ht_in{rep}h{h}",
                                name=f"ht_in{rep}h{h}") for h in range(2)]
        ht_out = [self.dram.tile([NCORES * D, HS], F16,
                                 tag=f"ht_out{rep}h{h}",
                                 name=f"ht_out{rep}h{h}",
                                 addr_space="Shared") for h in range(2)]
        hn = []
        for t in range(TLOC):
            ht = self.hp.tile([P, D], F32, tag=f"h{t % 2}",
                              name=f"h{t}_{rep}")
            nc.gpsimd.indirect_dma_start(
                out=ht[:], out_offset=None, in_=self.emb[:, :],
                in_offset=bass.IndirectOffsetOnAxis(
                    ap=self.x_sb[:, t:t + 1], axis=0),
            )
            sq = self.scratch.tile([P, D], F32, tag="sq",
                                   name=f"sq{t}_{rep}")
            ss = self.stats.tile([P, 1], F32, tag=f"ss{t}",
                                 name=f"ss{t}_{rep}")
            nc.scalar.activation(
                out=sq[:], in_=ht[:],
                func=mybir.ActivationFunctionType.Square, accum_out=ss[:])
            sd = self.stats.tile([P, 1], F32, tag=f"sd{t}",
                                 name=f"sd{t}_{rep}")
            nc.scalar.activation(
                out=sd[:], in_=ss[:],
                func=mybir.ActivationFunctionType.Sqrt,
                bias=self.eps_t[:], scale=1.0 / D)
            rv = self.stats.tile([P, 1], F32, tag=f"rv{t}",
                                 name=f"rv{t}_{rep}")
            nc.vector.reciprocal(rv[:], sd[:])
            hh = self.hp.tile([P, D], F16, tag=f"hn{t}", name=f"hn{t}_{rep}")
            # Pool engine: keeps DVE free for the surrounding body's copies
            nc.gpsimd.tensor_scalar_mul(out=hh[:], in0=ht[:], scalar1=rv[:])
            hn.append(hh)
        self.state[rep] = dict(ht_in=ht_in, ht_out=ht_out, hn=hn)

    # ---- head part B: h^T transposes + gather issue (PE/Pool/DVE/SP) ----
    def head_gather(self, rep):
        nc = self.nc
        st = self.state[rep]
        hn = st.pop("hn")
        ht_in = st["ht_in"]
        hTr = []
        for dc in range(DC):
            pt = self.pst.tile([P, SL], F16, tag="pt", name=f"pt{dc}_{rep}")
            for t in range(TLOC):
                nc.tensor.transpose(
                    pt[:, t * P:(t + 1) * P],
                    in_=hn[t][:, dc * P:(dc + 1) * P],
                    identity=self.ident_h[:])
            htr = self.htp.tile([P, SL], F16, tag=f"htr{dc}",
                                name=f"htr{dc}_{rep}")
            # GPSIMD cannot read PSUM -- DVE only here
            nc.vector.tensor_scalar_mul(
                out=htr[:], in0=pt[:], scalar1=self.w_cols[:, dc:dc + 1])
            hTr.append(htr)
            nc.sync.dma_start(
                out=ht_in[0][dc * P:(dc + 1) * P, :], in_=htr[:, 0:HS])
            nc.sync.dma_start(
                out=ht_in[1][dc * P:(dc + 1) * P, :], in_=htr[:, HS:SL])
        if MODE == "full":
            for h in range(2):
                nc.gpsimd.collective_compute(
                    "AllGather", mybir.AluOpType.bypass,
                    replica_groups=[list(range(NCORES))],
                    ins=[ht_in[h][:].opt()],
                    outs=[st["ht_out"][h][:].opt()])
        st["hTr"] = hTr

    def body(self, rep, reps):
        nc, tc = self.nc, self.tc
        st = self.state[rep]
        ht_in, ht_out = st["ht_in"], st["ht_out"]
        hTr = st["hTr"]

        # ---- q'' = h @ (Wq^T Wk) ----
        q2t = []
        with tc.tile_pool(name="psq", bufs=2, space="PSUM") as psq:
            for mo in range(DC):
                pp = psq.tile([P, SL], F32, tag="pp", name=f"pp{mo}_{rep}")
                for dc in range(DC):
                    nc.tensor.matmul(
                        pp[:], self.wm_sb[:, dc, mo * P:(mo + 1) * P],
                        hTr[dc][:], start=(dc == 0), stop=(dc == DC - 1))
                xt = self.q2p.tile([P, SL], F16, tag=f"q2t{mo}",
                                   name=f"q2t{mo}_{rep}")
                nc.vector.tensor_copy(xt[:], pp[:])
                q2t.append(xt)

        # ---- scores + flash softmax + row-major h production ----
        nm = [self.ostats.tile([P, JC], F32, tag=f"nm{t}", name=f"nm{t}_{rep}")
              for t in range(TLOC)]
        rs = [self.ostats.tile([P, JC], F32, tag=f"rs{t}", name=f"rs{t}_{rep}")
              for t in range(TLOC)]
        aT = [self.atp.tile([P, TLOC, P], F16, tag=f"aT{jb}",
                            name=f"aT{jb}_{rep}") for jb in range(JB)]
        hrm = [None] * JB
        with tc.tile_pool(name="ep", bufs=1) as ep:
            e = [ep.tile([P, S], F16, tag=f"e{t}", name=f"e{t}_{rep}")
                 for t in range(TLOC)]
            with (
                tc.tile_pool(name="kcp", bufs=2) as kcp,
                tc.tile_pool(name="pss", bufs=4, space="PSUM") as pss,
                tc.tile_pool(name="pth", bufs=2, space="PSUM") as pth,
            ):
                for jc in range(JC):
                    kc = kcp.tile([P, DC, SL], F16, tag="kc",
                                  name=f"kc{jc}_{rep}")
                    for h in range(2):
                        src = (ht_in[h][:, :] if MODE == "noag"
                               else ht_out[h][jc * D:(jc + 1) * D, :])
                        nc.sync.dma_start(
                            out=kc[:, :, h * HS:(h + 1) * HS],
                            in_=src.rearrange("(a b) c -> b a c", b=P))
                    for t in range(TLOC):
                        ps = pss.tile([P, SL], F32, tag="ps",
                                      name=f"ps{jc}_{t}_{rep}")
                        for dc in range(DC):
                            nc.tensor.matmul(
                                ps[:], q2t[dc][:, t * P:(t + 1) * P],
                                kc[:, dc, :],
                                start=(dc == 0), stop=(dc == DC - 1))
                        nc.vector.reduce_max(
                            out=nm[t][:, jc:jc + 1], in_=ps[:],
                            axis=mybir.AxisListType.X, negate=True)
                        nc.scalar.activation(
                            out=e[t][:, jc * SL:(jc + 1) * SL], in_=ps[:],
                            func=mybir.ActivationFunctionType.Exp,
                            bias=nm[t][:, jc:jc + 1], scale=1.0,
                            accum_out=rs[t][:, jc:jc + 1])
                    # transpose this chunk of h^T into row-major h while
                    # it's in SBUF (value-side operand of attn@h)
                    for c2 in range(TLOC):
                        jb = jc * TLOC + c2
                        ph = pth.tile([P, D], F16, tag="ph",
                                      name=f"ph{jb}_{rep}")
                        for dc in range(DC):
                            nc.tensor.transpose(
                                ph[:, dc * P:(dc + 1) * P],
                                in_=kc[:, dc, c2 * P:(c2 + 1) * P],
                                identity=self.ident_h[:])
                        hm = self.hrmp.tile([P, D], F16, tag=f"hrm{jb}",
                                            name=f"hrm{jb}_{rep}")
                        # GPSIMD cannot read PSUM: 3 copies DVE, 1 ACT
                        if c2 == 3:
                            nc.scalar.activation(
                                out=hm[:], in_=ph[:],
                                func=mybir.ActivationFunctionType.Copy)
                        else:
                            nc.vector.tensor_copy(hm[:], ph[:])
                        hrm[jb] = hm

            # ---- fixup factors: exp(m_c - m_g)/rowsum, then e *= g ----
            with tc.tile_pool(name="fxp", bufs=1) as fxp:
                for t in range(TLOC):
                    gneg = fxp.tile([P, 1], F32, tag=f"gneg{t}",
                                    name=f"gneg{t}_{rep}")
                    nc.vector.tensor_reduce(
                        out=gneg[:], in_=nm[t][:],
                        axis=mybir.AxisListType.X, op=mybir.AluOpType.min)
                    f = fxp.tile([P, JC], F32, tag=f"f{t}",
                                 name=f"f{t}_{rep}")
                    nc.scalar.activation(
                        out=f[:], in_=nm[t][:],
                        func=mybir.ActivationFunctionType.Exp,
                        bias=gneg[:], scale=-1.0)
                    wr = fxp.tile([P, JC], F32, tag=f"wr{t}",
                                  name=f"wr{t}_{rep}")
                    nc.vector.tensor_tensor(
                        out=wr[:], in0=rs[t][:], in1=f[:],
                        op=mybir.AluOpType.mult)
                    rowsum = fxp.tile([P, 1], F32, tag=f"rsum{t}",
                                      name=f"rsum{t}_{rep}")
                    nc.vector.reduce_sum(
                        out=rowsum[:], in_=wr[:], axis=mybir.AxisListType.X)
                    rinv = fxp.tile([P, 1], F32, tag=f"rinv{t}",
                                    name=f"rinv{t}_{rep}")
                    nc.vector.reciprocal(rinv[:], rowsum[:])
                    g = fxp.tile([P, JC], F32, tag=f"g{t}",
                                 name=f"g{t}_{rep}")
                    nc.vector.tensor_scalar_mul(
                        out=g[:], in0=f[:], scalar1=rinv[:])
                    for jc in range(JC):
                        nc.vector.tensor_scalar_mul(
                            out=e[t][:, jc * SL:(jc + 1) * SL],
                            in0=e[t][:, jc * SL:(jc + 1) * SL],
                            scalar1=g[:, jc:jc + 1])

            # next rep's head part A -- runs on ACT/DVE/Pool while this
            # rep's transposes and attn@h occupy the PE
            if rep + 1 < reps:
                self.head(rep + 1)

            # ---- aT transposes ----
            with tc.tile_pool(name="ptp", bufs=2, space="PSUM") as ptp:
                for jb in range(JB):
                    pt2 = ptp.tile([P, TLOC, P], F16, tag="pt2",
                                   name=f"pt2{jb}_{rep}")
                    for t in range(TLOC):
                        nc.tensor.transpose(
                            pt2[:, t, :],
                            in_=e[t][:, jb * P:(jb + 1) * P],
                            identity=self.ident_h[:])
                    nc.vector.tensor_copy(aT[jb][:], pt2[:])

        # next rep's head part B: h^T transposes on PE + AllGather issue --
        # the gather's wire time hides under this rep's attn@h + final
        if rep + 1 < reps:
            self.head_gather(rep + 1)

        # ---- attn @ h (output d-major), two passes of 4 PSUM banks ----
        ohT = [None] * DC
        with (
            tc.tile_pool(name="wvp", bufs=1) as wvp,
            tc.tile_pool(name="pvp", bufs=1, space="PSUM") as pvp,
        ):
            wv_sb = wvp.tile([P, DC, D], F16, name=f"wv_sb_{rep}")
            nc.scalar.dma_start(
                out=wv_sb[:],
                in_=self.wvt.ap().rearrange("(a b) c -> b a c", b=P))
            for dcg in range(2):
                po = [pvp.tile([P, TLOC, P], F32, tag=f"po{i}",
                               name=f"po{i}_{dcg}_{rep}") for i in range(4)]
                for jb in range(JB):
                    for i in range(4):
                        dc = dcg * 4 + i
                        nc.tensor.matmul(
                            po[i][:], hrm[jb][:, dc * P:(dc + 1) * P],
                            aT[jb][:],
                            start=(jb == 0), stop=(jb == JB - 1))
                for i in range(4):
                    dc = dcg * 4 + i
                    oh = self.ohp.tile([P, TLOC, P], F16, tag=f"ohT{dc}",
                                       name=f"ohT{dc}_{rep}")
                    nc.vector.tensor_copy(oh[:], po[i][:])
                    ohT[dc] = oh

            # ---- out = silu(out_h @ Wv^T) ----
            with (
                tc.tile_pool(name="outp", bufs=2) as outp,
                tc.tile_pool(name="pso", bufs=1, space="PSUM") as pso,
            ):
                for t in range(TLOC):
                    op = pso.tile([P, 2, 512], F32, tag="op",
                                  name=f"op{t}_{rep}")
                    for oh2 in range(2):
                        for dc in range(DC):
                            nc.tensor.matmul(
                                op[:, oh2, :], ohT[dc][:, t, :],
                                wv_sb[:, dc, oh2 * 512:(oh2 + 1) * 512],
                                start=(dc == 0), stop=(dc == DC - 1))
                    ot = outp.tile([P, 2, 512], F32, tag="ot",
                                   name=f"ot{t}_{rep}")
                    nc.scalar.activation(
                        out=ot[:], in_=op[:],
                        func=(mybir.ActivationFunctionType.Copy if NOSILU
                              else mybir.ActivationFunctionType.Silu))
                    nc.sync.dma_start(
                        out=self.out_loc[t * P:(t + 1) * P, :], in_=ot[:])


def kernel(x, emb, norm_w, Wq, Wk, Wv):
    if "nc" not in _cache:
        _cache["nc"] = build()
    nc = _cache["nc"]

    x = np.asarray(x).reshape(S).astype(np.int32)
    emb = np.ascontiguousarray(np.asarray(emb, dtype=np.float32))
    norm_w = np.ascontiguousarray(np.asarray(norm_w, dtype=np.float32))
    wm = np.ascontiguousarray(
        (np.asarray(Wq, np.float32).T @ np.asarray(Wk, np.float32))
        .astype(np.float16))
    wvt = np.ascontiguousarray(np.asarray(Wv, dtype=np.float16).T)

    in_maps = []
    for c in range(NCORES):
        in_maps.append({
            "x_loc": x[c * SL:(c + 1) * SL].reshape(SL, 1).copy(),
            "emb": emb, "norm_w": norm_w,
            "wm": wm, "wvt": wvt,
        })
    res = run_bass_kernel_spmd(nc, in_maps, core_ids=list(range(NCORES)),
                               **_cache.get("run_kwargs", {}))
    _cache["last_result"] = res
    out = np.concatenate([res.results[c]["out_loc"] for c in range(NCORES)],
                         axis=0)
    return out


# revision 50
# speedup vs baseline: 2.1188x; 1.2711x over previous
"""MiniTransformerBlock on 8 TRN2 NeuronCores (Bass/Tile), sequence-parallel.

Reference computation (S=4096, D=1024, V=32000):
    h = emb[x]                                  # [S, D]
    h = h * rsqrt(mean(h^2, -1) + eps) * norm_w # RMSNorm
    q, k, v = h @ Wq.T, h @ Wk.T, h @ Wv.T
    out = silu(softmax(q @ k.T) @ v)            # [S, D]  (no scale, no mask)

v4 ("hT-gather", software-pipelined). Key ideas vs the v2 baseline
(258 us):
  - softmax(q k^T) v = softmax((h Wq^T Wk) h^T) h Wv^T: gather ONLY the
    normalized h^T (8 MB fp16) instead of k^T + v (16 MB) -- HW A/B
    showed the two serialized v2 AllGathers exposed ~134 us.
  - flash softmax: per-512-chunk max + exp straight from PSUM into fp16,
    global fixup exp(m_c - m_g)/rowsum applied as one per-partition
    scalar per chunk.  No 8 MB f32 score buffer, no serial exp phase.
  - row-major h (for attn@h) is produced by PE-transposing the kc score
    tiles while they are in SBUF -- no extra HBM traffic.  (The DMA
    XBAR transpose is bit-correct in isolation but races with the
    collectives' SDMA fabric when reps pipeline, so it is not used.)
  - software-pipelined emission: rep r+1's head (embedding gather,
    RMSNorm on ACT/Pool, h^T transposes, AllGather issue) is emitted in
    the middle of rep r's body so the gather wire time hides under
    rep r's attn@h and rep r+1's projections.
  - weights live in SBUF (loaded once per call); all PSUM->SBUF copies
    on DVE/Pool so ACT only runs exp/silu; score chunks are 512 wide to
    halve per-instruction overheads.
"""

import os

import numpy as np

import concourse.bacc as bacc
import concourse.bass as bass
import concourse.tile as tile
from concourse import mybir
from concourse.bass_utils import run_bass_kernel_spmd
from concourse.masks import make_identity

P = 128
S = 4096
D = 1024
V = 32000
NCORES = 8
SL = S // NCORES          # 512 local rows
TLOC = SL // P            # 4 local row tiles
DC = D // P               # 8 feature chunks
JC = NCORES               # 8 key chunks of 512 (one per source core)
JB = S // P               # 32 key row blocks
HS = SL // 2              # 256 seq half (gather split granularity)
F32 = mybir.dt.float32
F16 = mybir.dt.float16
EPS = float(np.finfo(np.float32).eps)

_cache = {}

MODE = os.environ.get("BASS_MODE", "full")   # full | noag
REPS = int(os.environ.get("BASS_REPS", "1"))
NOSILU = os.environ.get("BASS_NOSILU", "0") == "1"  # CoreSim lacks Silu


def build(reps=None):
    if reps is None:
        reps = REPS
    nc = bacc.Bacc("TRN2", target_bir_lowering=False, debug=False,
                   num_devices=NCORES)

    x_loc = nc.dram_tensor("x_loc", [SL, 1], mybir.dt.int32, kind="ExternalInput")
    emb = nc.dram_tensor("emb", [V, D], F32, kind="ExternalInput")
    norm_w = nc.dram_tensor("norm_w", [D], F32, kind="ExternalInput")
    # host preps: wm[d,o] = (Wq.T @ Wk)[d,o];  wvt[d,o] = Wv[o,d]
    wm = nc.dram_tensor("wm", [D, D], F16, kind="ExternalInput")
    wvt = nc.dram_tensor("wvt", [D, D], F16, kind="ExternalInput")
    out_loc = nc.dram_tensor("out_loc", [SL, D], F32, kind="ExternalOutput")

    with tile.TileContext(nc) as tc:
        Pipe(nc, tc, x_loc, emb, norm_w, wm, wvt, out_loc).emit(reps)
    nc.compile()
    return nc


class Pipe:
    def __init__(self, nc, tc, x_loc, emb, norm_w, wm, wvt, out_loc):
        self.nc = nc
        self.tc = tc
        self.x_loc = x_loc
        self.emb = emb
        self.norm_w = norm_w
        self.wm = wm
        self.wvt = wvt
        self.out_loc = out_loc
        self.state = {}

    def emit(self, reps):
        nc, tc = self.nc, self.tc
        with (
            tc.tile_pool(name="const", bufs=1) as const,
            tc.tile_pool(name="dram", bufs=1, space="DRAM") as dram,
            tc.tile_pool(name="hp", bufs=1) as hp,
            tc.tile_pool(name="scratch", bufs=1) as scratch,
            tc.tile_pool(name="stats", bufs=1) as stats,
            tc.tile_pool(name="htp", bufs=1) as htp,
            tc.tile_pool(name="pst", bufs=2, space="PSUM") as pst,
            tc.tile_pool(name="q2p", bufs=1) as q2p,
            tc.tile_pool(name="ostats", bufs=1) as ostats,
            tc.tile_pool(name="atp", bufs=1) as atp,
            tc.tile_pool(name="hrmp", bufs=1) as hrmp,
            tc.tile_pool(name="ohp", bufs=1) as ohp,
        ):
            self.dram = dram
            self.hp = hp
            self.scratch = scratch
            self.stats = stats
            self.htp = htp
            self.pst = pst
            self.q2p = q2p
            self.ostats = ostats
            self.atp = atp
            self.hrmp = hrmp
            self.ohp = ohp

            ident_f = const.tile([P, P], F32)
            make_identity(nc, ident_f[:])
            self.ident_h = const.tile([P, P], F16)
            nc.vector.tensor_copy(self.ident_h[:], ident_f[:])
            self.eps_t = const.tile([P, 1], F32)
            nc.vector.memset(self.eps_t[:], EPS)
            self.w_cols = const.tile([P, DC], F32)
            nc.sync.dma_start(
                out=self.w_cols[:],
                in_=self.norm_w.ap().rearrange("(a b) -> b a", b=P))
            self.x_sb = const.tile([P, TLOC], mybir.dt.int32)
            nc.sync.dma_start(
                out=self.x_sb[:],
                in_=self.x_loc.ap().rearrange("(a b) c -> b (a c)", b=P))
            # q-path weight (Wq.T @ Wk, host-premultiplied) resident in SBUF
            self.wm_sb = const.tile([P, DC, D], F16)
            nc.sync.dma_start(
                out=self.wm_sb[:],
                in_=self.wm.ap().rearrange("(a b) c -> b a c", b=P))

            self.head(0)
            self.head_gather(0)
            for rep in range(reps):
                self.body(rep, reps)

    # ---- head part A: emb gather + RMSNorm (ACT/DVE/Pool only) ----
    def head(self, rep):
        nc = self.nc
        ht_in = [self.dram.tile([D, HS], F16, tag=f"ht_in{rep}h{h}",
                                name=f"ht_in{rep}h{h}") for h in range(2)]
        ht_out = [self.dram.tile([NCORES * D, HS], F16,
                                 tag=f"ht_out{rep}h{h}",
                                 name=f"ht_out{rep}h{h}",
                                 addr_space="Shared") for h in range(2)]
        hn = []
        for t in range(TLOC):
            ht = self.hp.tile([P, D], F32, tag=f"h{t % 2}",
                              name=f"h{t}_{rep}")
            nc.gpsimd.indirect_dma_start(
                out=ht[:], out_offset=None, in_=self.emb[:, :],
                in_offset=bass.IndirectOffsetOnAxis(
                    ap=self.x_sb[:, t:t + 1], axis=0),
            )
            sq = self.scratch.tile([P, D], F32, tag="sq",
                                   name=f"sq{t}_{rep}")
            ss = self.stats.tile([P, 1], F32, tag=f"ss{t}",
                                 name=f"ss{t}_{rep}")
            nc.scalar.activation(
                out=sq[:], in_=ht[:],
                func=mybir.ActivationFunctionType.Square, accum_out=ss[:])
            sd = self.stats.tile([P, 1], F32, tag=f"sd{t}",
                                 name=f"sd{t}_{rep}")
            nc.scalar.activation(
                out=sd[:], in_=ss[:],
                func=mybir.ActivationFunctionType.Sqrt,
                bias=self.eps_t[:], scale=1.0 / D)
            rv = self.stats.tile([P, 1], F32, tag=f"rv{t}",
                                 name=f"rv{t}_{rep}")
            nc.vector.reciprocal(rv[:], sd[:])
            hh = self.hp.tile([P, D], F16, tag=f"hn{t}", name=f"hn{t}_{rep}")
            # Pool engine: keeps DVE free for the surrounding body's copies
            nc.gpsimd.tensor_scalar_mul(out=hh[:], in0=ht[:], scalar1=rv[:])
            hn.append(hh)
        self.state[rep] = dict(ht_in=ht_in, ht_out=ht_out, hn=hn)

    # ---- head part B: h^T transposes + gather issue (PE/Pool/DVE/SP) ----
    def head_gather(self, rep):
        nc = self.nc
        st = self.state[rep]
        hn = st.pop("hn")
        ht_in = st["ht_in"]
        hTr = []
        for dc in range(DC):
            pt = self.pst.tile([P, SL], F16, tag="pt", name=f"pt{dc}_{rep}")
            for t in range(TLOC):
                nc.tensor.transpose(
                    pt[:, t * P:(t + 1) * P],
                    in_=hn[t][:, dc * P:(dc + 1) * P],
                    identity=self.ident_h[:])
            htr = self.htp.tile([P, SL], F16, tag=f"htr{dc}",
                                name=f"htr{dc}_{rep}")
            # GPSIMD cannot read PSUM -- DVE only here
            nc.vector.tensor_scalar_mul(
                out=htr[:], in0=pt[:], scalar1=self.w_cols[:, dc:dc + 1])
            hTr.append(htr)
            nc.sync.dma_start(
                out=ht_in[0][dc * P:(dc + 1) * P, :], in_=htr[:, 0:HS])
            nc.sync.dma_start(
                out=ht_in[1][dc * P:(dc + 1) * P, :], in_=htr[:, HS:SL])
        if MODE == "full":
            for h in range(2):
                nc.gpsimd.collective_compute(
                    "AllGather", mybir.AluOpType.bypass,
                    replica_groups=[list(range(NCORES))],
                    ins=[ht_in[h][:].opt()],
                    outs=[st["ht_out"][h][:].opt()])
        st["hTr"] = hTr

    def body(self, rep, reps):
        nc, tc = self.nc, self.tc
        st = self.state[rep]
        ht_in, ht_out = st["ht_in"], st["ht_out"]
        hTr = st["hTr"]

        # ---- q'' = h @ (Wq^T Wk) ----
        q2t = []
        with tc.tile_pool(name="psq", bufs=2, space="PSUM") as psq:
            for mo in range(DC):
                pp = psq.tile([P, SL], F32, tag="pp", name=f"pp{mo}_{rep}")
                for dc in range(DC):
                    nc.tensor.matmul(
                        pp[:], self.wm_sb[:, dc, mo * P:(mo + 1) * P],
                        hTr[dc][:], start=(dc == 0), stop=(dc == DC - 1))
                xt = self.q2p.tile([P, SL], F16, tag=f"q2t{mo}",
                                   name=f"q2t{mo}_{rep}")
                nc.vector.tensor_copy(xt[:], pp[:])
                q2t.append(xt)

        # ---- scores + flash softmax + row-major h production ----
        nm = [self.ostats.tile([P, JC], F32, tag=f"nm{t}", name=f"nm{t}_{rep}")
              for t in range(TLOC)]
        rs = [self.ostats.tile([P, JC], F32, tag=f"rs{t}", name=f"rs{t}_{rep}")
              for t in range(TLOC)]
        aT = [self.atp.tile([P, TLOC, P], F16, tag=f"aT{jb}",
                            name=f"aT{jb}_{rep}") for jb in range(JB)]
        hrm = [None] * JB
        with tc.tile_pool(name="ep", bufs=1) as ep:
            e = [ep.tile([P, S], F16, tag=f"e{t}", name=f"e{t}_{rep}")
                 for t in range(TLOC)]
            with (
                tc.tile_pool(name="kcp", bufs=2) as kcp,
                tc.tile_pool(name="pss", bufs=4, space="PSUM") as pss,
                tc.tile_pool(name="pth", bufs=2, space="PSUM") as pth,
            ):
                for jc in range(JC):
                    kc = kcp.tile([P, DC, SL], F16, tag="kc",
                                  name=f"kc{jc}_{rep}")
                    for h in range(2):
                        for q2 in range(2):
                            cs = slice(q2 * P, (q2 + 1) * P)
                            src = (ht_in[h][:, cs] if MODE == "noag"
                                   else ht_out[h][jc * D:(jc + 1) * D, cs])
                            o0 = h * HS + q2 * P
                            nc.sync.dma_start(
                                out=kc[:, :, o0:o0 + P],
                                in_=src.rearrange("(a b) c -> b a c", b=P))
                    for t in range(TLOC):
                        ps = pss.tile([P, SL], F32, tag="ps",
                                      name=f"ps{jc}_{t}_{rep}")
                        for dc in range(DC):
                            nc.tensor.matmul(
                                ps[:], q2t[dc][:, t * P:(t + 1) * P],
                                kc[:, dc, :],
                                start=(dc == 0), stop=(dc == DC - 1))
                        nc.vector.reduce_max(
                            out=nm[t][:, jc:jc + 1], in_=ps[:],
                            axis=mybir.AxisListType.X, negate=True)
                        nc.scalar.activation(
                            out=e[t][:, jc * SL:(jc + 1) * SL], in_=ps[:],
                            func=mybir.ActivationFunctionType.Exp,
                            bias=nm[t][:, jc:jc + 1], scale=1.0,
                            accum_out=rs[t][:, jc:jc + 1])
                    # transpose this chunk of h^T into row-major h while
                    # it's in SBUF (value-side operand of attn@h)
                    for c2 in range(TLOC):
                        jb = jc * TLOC + c2
                        ph = pth.tile([P, D], F16, tag="ph",
                                      name=f"ph{jb}_{rep}")
                        for dc in range(DC):
                            nc.tensor.transpose(
                                ph[:, dc * P:(dc + 1) * P],
                                in_=kc[:, dc, c2 * P:(c2 + 1) * P],
                                identity=self.ident_h[:])
                        hm = self.hrmp.tile([P, D], F16, tag=f"hrm{jb}",
                                            name=f"hrm{jb}_{rep}")
                        # GPSIMD cannot read PSUM: 3 copies DVE, 1 ACT
                        if c2 == 3:
                            nc.scalar.activation(
                                out=hm[:], in_=ph[:],
                                func=mybir.ActivationFunctionType.Copy)
                        else:
                            nc.vector.tensor_copy(hm[:], ph[:])
                        hrm[jb] = hm

            # ---- fixup factors: exp(m_c - m_g)/rowsum, then e *= g ----
            with tc.tile_pool(name="fxp", bufs=1) as fxp:
                for t in range(TLOC):
                    gneg = fxp.tile([P, 1], F32, tag=f"gneg{t}",
                                    name=f"gneg{t}_{rep}")
                    nc.vector.tensor_reduce(
                        out=gneg[:], in_=nm[t][:],
                        axis=mybir.AxisListType.X, op=mybir.AluOpType.min)
                    f = fxp.tile([P, JC], F32, tag=f"f{t}",
                                 name=f"f{t}_{rep}")
                    nc.scalar.activation(
                        out=f[:], in_=nm[t][:],
                        func=mybir.ActivationFunctionType.Exp,
                        bias=gneg[:], scale=-1.0)
                    wr = fxp.tile([P, JC], F32, tag=f"wr{t}",
                                  name=f"wr{t}_{rep}")
                    nc.vector.tensor_tensor(
                        out=wr[:], in0=rs[t][:], in1=f[:],
                        op=mybir.AluOpType.mult)
                    rowsum = fxp.tile([P, 1], F32, tag=f"rsum{t}",
                                      name=f"rsum{t}_{rep}")
                    nc.vector.reduce_sum(
                        out=rowsum[:], in_=wr[:], axis=mybir.AxisListType.X)
                    rinv = fxp.tile([P, 1], F32, tag=f"rinv{t}",
                                    name=f"rinv{t}_{rep}")
                    nc.vector.reciprocal(rinv[:], rowsum[:])
                    g = fxp.tile([P, JC], F32, tag=f"g{t}",
                                 name=f"g{t}_{rep}")
                    nc.vector.tensor_scalar_mul(
                        out=g[:], in0=f[:], scalar1=rinv[:])
                    for jc in range(JC):
                        nc.vector.tensor_scalar_mul(
                            out=e[t][:, jc * SL:(jc + 1) * SL],
                            in0=e[t][:, jc * SL:(jc + 1) * SL],
                            scalar1=g[:, jc:jc + 1])

            # next rep's head part A -- runs on ACT/DVE/Pool while this
            # rep's transposes and attn@h occupy the PE
            if rep + 1 < reps:
                self.head(rep + 1)

            # ---- aT transposes ----
            with tc.tile_pool(name="ptp", bufs=2, space="PSUM") as ptp:
                for jb in range(JB):
                    pt2 = ptp.tile([P, TLOC, P], F16, tag="pt2",
                                   name=f"pt2{jb}_{rep}")
                    for t in range(TLOC):
                        nc.tensor.transpose(
                            pt2[:, t, :],
                            in_=e[t][:, jb * P:(jb + 1) * P],
                            identity=self.ident_h[:])
                    nc.vector.tensor_copy(aT[jb][:], pt2[:])

        # next rep's head part B: h^T transposes on PE + AllGather issue --
        # the gather's wire time hides under this rep's attn@h + final
        if rep + 1 < reps:
            self.head_gather(rep + 1)

        # ---- attn @ h (output d-major), two passes of 4 PSUM banks ----
        ohT = [None] * DC
        with (
            tc.tile_pool(name="wvp", bufs=1) as wvp,
            tc.tile_pool(name="pvp", bufs=1, space="PSUM") as pvp,
        ):
            wv_sb = wvp.tile([P, DC, D], F16, name=f"wv_sb_{rep}")
            nc.scalar.dma_start(
                out=wv_sb[:],
                in_=self.wvt.ap().rearrange("(a b) c -> b a c", b=P))
            for dcg in range(2):
                po = [pvp.tile([P, TLOC, P], F32, tag=f"po{i}",
                               name=f"po{i}_{dcg}_{rep}") for i in range(4)]
                for jb in range(JB):
                    for i in range(4):
                        dc = dcg * 4 + i
                        nc.tensor.matmul(
                            po[i][:], hrm[jb][:, dc * P:(dc + 1) * P],
                            aT[jb][:],
                            start=(jb == 0), stop=(jb == JB - 1))
                for i in range(4):
                    dc = dcg * 4 + i
                    oh = self.ohp.tile([P, TLOC, P], F16, tag=f"ohT{dc}",
                                       name=f"ohT{dc}_{rep}")
                    nc.vector.tensor_copy(oh[:], po[i][:])
                    ohT[dc] = oh

            # ---- out = silu(out_h @ Wv^T) ----
            with (
                tc.tile_pool(name="outp", bufs=2) as outp,
                tc.tile_pool(name="pso", bufs=1, space="PSUM") as pso,
            ):
                for t in range(TLOC):
                    op = pso.tile([P, 2, 512], F32, tag="op",
                                  name=f"op{t}_{rep}")
                    for oh2 in range(2):
                        for dc in range(DC):
                            nc.tensor.matmul(
                                op[:, oh2, :], ohT[dc][:, t, :],
                                wv_sb[:, dc, oh2 * 512:(oh2 + 1) * 512],
                                start=(dc == 0), stop=(dc == DC - 1))
                    ot = outp.tile([P, 2, 512], F32, tag="ot",
                                   name=f"ot{t}_{rep}")
                    nc.scalar.activation(
                        out=ot[:], in_=op[:],
                        func=(mybir.ActivationFunctionType.Copy if NOSILU
                              else mybir.ActivationFunctionType.Silu))
                    nc.sync.dma_start(
                        out=self.out_loc[t * P:(t + 1) * P, :], in_=ot[:])


def kernel(x, emb, norm_w, Wq, Wk, Wv):
    if "nc" not in _cache:
        _cache["nc"] = build()
    nc = _cache["nc"]

    x = np.asarray(x).reshape(S).astype(np.int32)
    emb = np.ascontiguousarray(np.asarray(emb, dtype=np.float32))
    norm_w = np.ascontiguousarray(np.asarray(norm_w, dtype=np.float32))
    wm = np.ascontiguousarray(
        (np.asarray(Wq, np.float32).T @ np.asarray(Wk, np.float32))
        .astype(np.float16))
    wvt = np.ascontiguousarray(np.asarray(Wv, dtype=np.float16).T)

    in_maps = []
    for c in range(NCORES):
        in_maps.append({
            "x_loc": x[c * SL:(c + 1) * SL].reshape(SL, 1).copy(),
            "emb": emb, "norm_w": norm_w,
            "wm": wm, "wvt": wvt,
        })
    res = run_bass_kernel_spmd(nc, in_maps, core_ids=list(range(NCORES)),
                               **_cache.get("run_kwargs", {}))
    _cache["last_result"] = res
    out = np.concatenate([res.results[c]["out_loc"] for c in range(NCORES)],
                         axis=0)
    return out
